# revision 18
# baseline (speedup 1.0000x reference)
"""Trainium2 Bass kernel for the attention+global-LN+MoE(top2)+global-LN block.

Strategy (8 NeuronCores):
  Launch A (fp8 e4m3 matmuls, DoubleRow where contraction >= 256): attention
      + W2 + residual, column-parallel over heads (3 heads/core, 2 samples x
      4 head-groups). The reference's raw [h,dh,N]->[N,h*dh] reshape maps
      head-group q onto view-rows [512q, 512q+512), so each core owns 512
      rows of its sample. Power-of-2 pre-scales keep every fp8 tensor out of
      the subnormal range: Q,K x32 (folded into the exp scale), V x16,
      softmax weights x64 (folded into 1/sum; removed in the O-copy), W2 x64.
      Emits y1' = 1024*y1 (fp32) + per-channel (sum, sumsq).
  Host: combines LN1 stats, applies the LN1 affine to y1 (fp64), computes
      the router gate, picks top-2 experts per sample, quantizes x1 and the
      selected experts' weights to fp8 (x1024 scale; gate value folded into
      proj weights).
  Launch B (fp8 DoubleRow): the 2 selected experts' fc+gelu+proj fused with
      the residual, software-pipelined so fc runs one 256-column pair ahead
      of proj (hides the gelu latency). Emits 1024*(moe+x1) + stats.
  Host: combines LN2 stats and applies the final affine while unsharding.
"""

import numpy as np
import ml_dtypes

import concourse.bass as bass
from concourse import bacc
import concourse.mybir as mybir
import concourse.tile as tile
from concourse.bass_utils import run_bass_kernel_spmd
from concourse.masks import make_identity

F32 = mybir.dt.float32
F8 = mybir.dt.float8e4
BF16 = mybir.dt.bfloat16
AF = mybir.ActivationFunctionType
AX = mybir.AxisListType
DR = mybir.MatmulPerfMode.DoubleRow

NP_F8 = ml_dtypes.float8_e4m3

B, N, D, E = 2, 2048, 768, 8
H = 4 * D            # 3072
NH = 12              # heads
DH = D // NH         # 64
TOP_K = 2
P = 128
ROWS = 512           # rows per core
HPC = 3              # heads per core
EPS = 1e-12
M_TOT = B * N * D
SQK = 32.0           # Q/K fp8 pre-scale
SV = 16.0            # V fp8 pre-scale
SW2 = 64.0           # W2 fp8 pre-scale
SCALE_A = SV * SW2   # launch A output scale: y1' = 1024*y1
EXP_SCALE = 1.0 / (SQK * SQK * float(np.sqrt(np.float32(N))))
SCALE = 1024.0       # MoE fp8 weight pre-scale; y2' = 1024*y2

N_CORES = 8


def _r(ap, pat, **kw):
    return ap.rearrange(pat, **kw)


# ---------------------------------------------------------------- launch A ---
def build_launch_a():
    nc = bacc.Bacc(None, target_bir_lowering=False, debug=False)
    xT = nc.declare_dram_parameter("xT", [4, P, 6, 512], F8, isOutput=False)
    w1qk = nc.declare_dram_parameter("w1qk", [P, 6, 384], F8, isOutput=False)
    b1qk = nc.declare_dram_parameter("b1qk", [P, 2 * HPC * DH], F32, isOutput=False)
    w1v = nc.declare_dram_parameter("w1v", [P, 6, 192], F8, isOutput=False)
    b1v = nc.declare_dram_parameter("b1v", [P, 2], F32, isOutput=False)
    w2 = nc.declare_dram_parameter("w2", [P, 6, D], F8, isOutput=False)
    xb = nc.declare_dram_parameter("xb", [P, 6, ROWS], F32, isOutput=False)
    y1T_out = nc.declare_dram_parameter("y1T", [D, ROWS], F32, isOutput=True)
    stats_out = nc.declare_dram_parameter("stats", [P, 12], F32, isOutput=True)

    o_dram = nc.dram_tensor("o_scratch", [ROWS, D], BF16)

    with tile.TileContext(nc) as tc:
        with (
            tc.tile_pool(name="const", bufs=1) as const,
            tc.tile_pool(name="persist", bufs=1) as persist,
            tc.tile_pool(name="small", bufs=4) as small,
        ):
            ident = const.tile([P, P], BF16)
            make_identity(nc, ident)
            ones_sb = const.tile([P, 8], F8)
            nc.vector.memset(ones_sb[:], 1.0)
            b1qk_sb = const.tile([P, 384], F32)
            nc.gpsimd.dma_start(out=b1qk_sb[:], in_=b1qk[:])
            b1v_sb = const.tile([P, 2], F32)
            nc.gpsimd.dma_start(out=b1v_sb[:], in_=b1v[:])

            qk_sb = persist.tile([P, 16, 384], F8)
            vt_sb = persist.tile([P, 2, N], F8)
            ovt_sb = persist.tile([P, 6, 512], F8)

            with (
                tc.tile_pool(name="xtp", bufs=1) as xtp,
                tc.tile_pool(name="psA", bufs=2, space="PSUM") as psA,
            ):
                w1qk_sb = xtp.tile([P, 6, 384], F8)
                nc.sync.dma_start(out=w1qk_sb[:], in_=w1qk[:])
                xT_c = []
                for f in range(4):
                    xt_t = xtp.tile([P, 6, 512], F8, tag=f"xt{f}",
                                    name=f"xt_t{f}")
                    nc.sync.dma_start(out=xt_t[:], in_=xT[f])
                    xT_c.append(xt_t)
                w1v_sb = xtp.tile([P, 6, 192], F8)
                nc.scalar.dma_start(out=w1v_sb[:], in_=w1v[:])

                w2_sb = persist.tile([P, 6, D], F8)
                xb_sb = persist.tile([P, 6, ROWS], F32)

                # ---- phase 1: Q,K = x @ W1[qk cols] -> [n(part), 384] -------
                for m in range(16):
                    c, mi = divmod(m, 4)
                    ps = psA.tile([P, 384], F32, tag="qk", bufs=3)
                    for kk, b in enumerate((0, 2, 4)):
                        nc.tensor.matmul(
                            ps[:],
                            xT_c[c][:, b:b + 2, mi * P:(mi + 1) * P],
                            w1qk_sb[:, b:b + 2, :],
                            start=(kk == 0),
                            stop=(kk == 2),
                            perf_mode=DR,
                        )
                    nc.vector.tensor_add(qk_sb[:, m, :], ps[:], b1qk_sb[:])

                # ---- phase 2: V^T = W1v^T @ x^T -> [dh(part) x 2, N] --------
                for mo in range(2):
                    mp = P if mo == 0 else 64
                    for f in range(4):
                        ps = psA.tile([P, 512], F32, tag="vt")
                        for kk, b in enumerate((0, 2, 4)):
                            nc.tensor.matmul(
                                ps[:mp],
                                w1v_sb[:, b:b + 2, mo * P: mo * P + mp],
                                xT_c[f][:, b:b + 2, :],
                                start=(kk == 0),
                                stop=(kk == 2),
                                perf_mode=DR,
                            )
                        nc.scalar.activation(
                            out=vt_sb[:mp, mo, f * 512:(f + 1) * 512],
                            in_=ps[:mp],
                            func=AF.Identity,
                            bias=b1v_sb[:mp, mo: mo + 1],
                        )

            # ---- phase 3: per-head scores/softmax/O -------------------------
            o_flat = _r(_r(o_dram[:], "a c -> (a c)"),
                        "(h d n) -> d h n", h=HPC, d=64)
            ov_c = []
            ovp_cm = tc.tile_pool(name="ovp", bufs=1)
            ovp = ovp_cm.__enter__()
            with (
                tc.tile_pool(name="op", bufs=1) as op,
                tc.tile_pool(name="psB", bufs=2, space="PSUM") as psB,
            ):
                nc.scalar.dma_start(out=w2_sb[:], in_=w2[:])
                o_sb = op.tile([64, HPC, N], BF16)
                # scores for all heads first, then softmax/O interleaved so
                # the PE never idles waiting on an exp
                ps_scs = []
                wtes = []
                for h in range(HPC):
                    ps_sc = psB.tile([64, 64], F32, tag=f"sc{h}",
                                     name=f"ps_sc{h}", bufs=1)
                    for mm in range(8):
                        m = 2 * mm
                        nc.tensor.matmul(
                            ps_sc[:],
                            qk_sb[:, m:m + 2, 192 + h * 64: 192 + (h + 1) * 64],
                            qk_sb[:, m:m + 2, h * 64:(h + 1) * 64],
                            start=(mm == 0),
                            stop=(mm == 7),
                            perf_mode=DR,
                        )
                    ps_scs.append(ps_sc)
                    # logits are small (|s|<4): exp without max subtraction
                    wte = small.tile([P, 64], F8, tag=f"wte{h}",
                                     name=f"wte{h}")
                    off = 64 if h == 1 else 0
                    if off == 0:
                        nc.scalar.activation(out=wte[0:64, :], in_=ps_sc[:],
                                             func=AF.Exp, scale=EXP_SCALE)
                    else:
                        # shift to partitions 64:128 (lane shift needs a DMA)
                        wte_tmp = small.tile([64, 64], F8, tag="wtetmp")
                        nc.scalar.activation(out=wte_tmp[:], in_=ps_sc[:],
                                             func=AF.Exp, scale=EXP_SCALE)
                        nc.sync.dma_start(out=wte[64:128, :], in_=wte_tmp[:])
                    wtes.append(wte)
                for h in range(HPC):
                    off = 64 if h == 1 else 0
                    vchunk = 0 if h < 2 else 1
                    wte = wtes[h]
                    ps_sm = psB.tile([64, 8], F32, tag="sm", bufs=2)
                    nc.tensor.matmul(
                        ps_sm[:],
                        wte[off:off + 64, :],
                        ones_sb[off:off + 64, :],
                        start=True,
                        stop=True,
                    )
                    rinv = small.tile([64, 1], F32, tag="rinv")
                    nc.vector.reciprocal(out=rinv[:], in_=ps_sm[:, 0:1])
                    for f in range(4):
                        ps_o = psB.tile([64, 512], F32, tag="o")
                        nc.tensor.matmul(
                            ps_o[:],
                            wte[off:off + 64, :],
                            vt_sb[off:off + 64, vchunk, f * 512:(f + 1) * 512],
                            start=True,
                            stop=True,
                        )
                        nc.scalar.activation(
                            out=o_sb[:, h, f * 512:(f + 1) * 512],
                            in_=ps_o[:], func=AF.Copy, scale=rinv[:, 0:1])
                    nc.sync.dma_start(out=o_flat[:, h, :], in_=o_sb[:, h, :])
                    # chunk a of the row-view depends only on heads <= a';
                    # issue its read as soon as the covering head is written
                    for a in ((0,) if h == 0 else (1,) if h == 1 else (2, 3)):
                        ov_t = ovp.tile([P, D], BF16, tag=f"ov{a}",
                                        name=f"ov_t{a}")
                        nc.sync.dma_start(out=ov_t[:],
                                          in_=o_dram[a * P:(a + 1) * P, :])
                        ov_c.append(ov_t)
                nc.gpsimd.dma_start(out=xb_sb[:], in_=xb[:])

            # ---- phase 5+6 share one PSUM pool (6 transpose + 2 matmul
            # banks = 8) so no pool-transition barrier separates them --------
            with (
                tc.tile_pool(name="yp", bufs=3) as yp,
                tc.tile_pool(name="ps56", bufs=1, space="PSUM") as ps56,
            ):
                ps_ts = [ps56.tile([P, 512], BF16, tag=f"ovt{bb}", name=f"ps_t{bb}")
                         for bb in range(6)]
                # chunks a0..a2 transpose as soon as they land; a3's
                # transposes interleave with the ovt copies
                for a in range(3):
                    for bb in range(6):
                        nc.tensor.transpose(
                            ps_ts[bb][:, a * P:(a + 1) * P],
                            ov_c[a][:, bb * P:(bb + 1) * P],
                            ident[:],
                        )
                for bb in range(6):
                    nc.tensor.transpose(
                        ps_ts[bb][:, 3 * P:4 * P],
                        ov_c[3][:, bb * P:(bb + 1) * P],
                        ident[:],
                    )
                    nc.scalar.copy(ovt_sb[:, bb, :], ps_ts[bb][:])

                stats_sb = small.tile([P, 6, 2], F32, tag="stats")
                for dc in range(6):
                    ps_y = ps56.tile([P, 512], F32, tag="y", bufs=2)
                    for kk, b in enumerate((0, 2, 4)):
                        nc.tensor.matmul(
                            ps_y[:],
                            w2_sb[:, b:b + 2, dc * P:(dc + 1) * P],
                            ovt_sb[:, b:b + 2, :],
                            start=(kk == 0),
                            stop=(kk == 2),
                            perf_mode=DR,
                        )
                    y_sb = yp.tile([P, 512], F32, tag="ytile")
                    nc.vector.tensor_add(y_sb[:], ps_y[:], xb_sb[:, dc, :])
                    nc.vector.reduce_sum(out=stats_sb[:, dc, 0:1], in_=y_sb[:],
                                         axis=AX.X)
                    sq = yp.tile([P, 512], F32, tag="sq")
                    nc.scalar.activation(out=sq[:], in_=y_sb[:],
                                         func=AF.Square,
                                         accum_out=stats_sb[:, dc, 1:2])
                    qeng = nc.sync if dc % 2 == 0 else nc.scalar
                    qeng.dma_start(
                        out=_r(y1T_out[:], "(po pi) n -> pi po n", pi=P)[:, dc, :],
                        in_=y_sb[:],
                    )
                nc.sync.dma_start(
                    out=stats_out[:],
                    in_=_r(stats_sb[:], "p a b -> p (a b)"),
                )
            ovp_cm.__exit__(None, None, None)
    nc.compile()
    return nc


# ---------------------------------------------------------------- launch B ---
def build_launch_b():
    nc = bacc.Bacc(None, target_bir_lowering=False, debug=False)
    x1f8 = nc.declare_dram_parameter("x1f8", [P, 6, 512], F8, isOutput=False)
    xb = nc.declare_dram_parameter("xb", [P, 6, 512], F32, isOutput=False)
    fcw = [nc.declare_dram_parameter(f"fcw{e}", [6, P, 6, 512], F8, isOutput=False)
           for e in range(2)]
    fcb = [nc.declare_dram_parameter(f"fcb{e}", [P, 24], F32, isOutput=False)
           for e in range(2)]
    pjw = [nc.declare_dram_parameter(f"pjw{e}", [12, P, 2, D], F8, isOutput=False)
           for e in range(2)]
    y2T_out = nc.declare_dram_parameter("y2T", [D, ROWS], F32, isOutput=True)
    stats_out = nc.declare_dram_parameter("stats", [P, 12], F32, isOutput=True)

    with tile.TileContext(nc) as tc:
        with (
            tc.tile_pool(name="const", bufs=1) as const,
            tc.tile_pool(name="wstream", bufs=3) as wstream,
            tc.tile_pool(name="pstream", bufs=3) as pstream,
            tc.tile_pool(name="hm", bufs=3) as hmp,
            tc.tile_pool(name="small", bufs=4) as small,
            tc.tile_pool(name="psacc", bufs=1, space="PSUM") as psacc,
            tc.tile_pool(name="pshm", bufs=2, space="PSUM") as pshm,
        ):
            x1f8_sb = const.tile([P, 6, 512], F8)
            nc.sync.dma_start(out=x1f8_sb[:], in_=x1f8[:])
            xb_sb = const.tile([P, 6, 512], F32)
            fcb_sb = [const.tile([P, 24], F32, tag=f"fcb{e}", name=f"fcb_sb{e}")
                      for e in range(2)]
            for e in range(2):
                nc.gpsimd.dma_start(out=fcb_sb[e][:], in_=fcb[e][:])

            # MoE: fp8 DoubleRow matmuls; fc runs one 256-col pair ahead of
            # proj so the gelu latency is hidden behind proj matmuls.
            ps_out = [psacc.tile([P, 512], F32, tag=f"acc{dc}",
                                 name=f"ps_out{dc}") for dc in range(6)]
            pairs = [(e, g, pp) for e in range(2) for g in range(6)
                     for pp in range(2)]
            fcw_t = None
            prev = None  # (pjw_t, hm_t, first)
            for i, (e, g, pp) in enumerate(pairs):
                if pp == 0:
                    fcw_t = wstream.tile([P, 6, 512], F8, tag="fcw")
                    nc.sync.dma_start(out=fcw_t[:], in_=fcw[e][g])
                pr = g * 2 + pp
                pjw_t = pstream.tile([P, 2, D], F8, tag="pjw")
                nc.gpsimd.dma_start(out=pjw_t[:], in_=pjw[e][pr])
                if i == 4:
                    nc.gpsimd.dma_start(out=xb_sb[:], in_=xb[:])
                hm_t = hmp.tile([P, 2, 512], F8, tag="hm")
                for j in range(2):
                    fo = g * 4 + pp * 2 + j
                    col0 = pp * 256 + j * 128
                    ps_h = pshm.tile([P, 512], F32, tag="h")
                    for kk, b in enumerate((0, 2, 4)):
                        nc.tensor.matmul(
                            ps_h[:],
                            fcw_t[:, b:b + 2, col0:col0 + 128],
                            x1f8_sb[:, b:b + 2, :],
                            start=(kk == 0),
                            stop=(kk == 2),
                            perf_mode=DR,
                        )
                    nc.scalar.activation(
                        out=hm_t[:, j, :], in_=ps_h[:],
                        func=AF.Gelu_apprx_tanh,
                        scale=1.0 / SCALE,
                        bias=fcb_sb[e][:, fo: fo + 1])
                if prev is not None:
                    p_pjw, p_hm, p_first = prev
                    for dc in range(6):
                        nc.tensor.matmul(
                            ps_out[dc][:], p_pjw[:, :, dc * P:(dc + 1) * P],
                            p_hm[:, :, :], start=p_first, stop=False,
                            perf_mode=DR,
                        )
                prev = (pjw_t, hm_t, i == 0)
            p_pjw, p_hm, p_first = prev
            for dc in range(6):
                nc.tensor.matmul(
                    ps_out[dc][:], p_pjw[:, :, dc * P:(dc + 1) * P],
                    p_hm[:, :, :], start=p_first, stop=True,
                    perf_mode=DR,
                )

            # y2' = 1024*(moe + x1) ; per-channel stats; LN2 on host
            stats_sb = small.tile([P, 6, 2], F32, tag="stats")
            for dc in range(6):
                y2 = hmp.tile([P, 512], F32, tag="y2")
                nc.vector.tensor_add(y2[:], ps_out[dc][:], xb_sb[:, dc, :])
                nc.vector.reduce_sum(out=stats_sb[:, dc, 0:1], in_=y2[:],
                                     axis=AX.X)
                sq = hmp.tile([P, 512], F32, tag="sq")
                nc.scalar.activation(out=sq[:], in_=y2[:], func=AF.Square,
                                     accum_out=stats_sb[:, dc, 1:2])
                qeng = nc.sync if dc % 2 == 0 else nc.scalar
                qeng.dma_start(
                    out=_r(y2T_out[:], "(po pi) n -> pi po n", pi=P)[:, dc, :],
                    in_=y2[:],
                )
            nc.sync.dma_start(
                out=stats_out[:],
                in_=_r(stats_sb[:], "p a b -> p (a b)"),
            )
    nc.compile()
    return nc


# ------------------------------------------------------------------- host ---
_CACHE = {}
PROFILE = False          # set True (e.g. from test.py) to capture NTFF timing
LAST_EXEC_NS = {}


def _get_nc(which):
    if which not in _CACHE:
        _CACHE[which] = build_launch_a() if which == "a" else build_launch_b()
    return _CACHE[which]


def _softmax_np(x):
    x = x - x.max()
    e = np.exp(x)
    return e / e.sum()


def _run(which, in_maps):
    kwargs = {}
    if PROFILE:
        kwargs = dict(trace=True)
    res = run_bass_kernel_spmd(_get_nc(which), in_maps, list(range(N_CORES)),
                               **kwargs)
    if res.exec_time_ns is not None:
        LAST_EXEC_NS[which] = res.exec_time_ns
    return res


def pack_po(a):
    """[K, F] -> [128, K//128, F] SBUF-layout pack (contiguous DMA)."""
    K_, F_ = a.shape
    return np.ascontiguousarray(
        a.reshape(K_ // P, P, F_).transpose(1, 0, 2))


def f8(a):
    return np.clip(np.asarray(a, np.float32), -224, 224).astype(NP_F8)


def pack_a_inputs(x, W1_w, W1_b, W2_w, W2_b):
    f32 = lambda a: np.ascontiguousarray(a, np.float32)
    xT_pk = []
    xb_pk = []
    for s in range(B):
        xTs = f8(x[s].T)
        xT_pk.append(np.stack([pack_po(xTs[:, c * 512:(c + 1) * 512])
                               for c in range(4)]))
        xb_pk.append(SCALE_A * (f32(x[s].T) + W2_b[:, None].astype(np.float32)))
    w2_pk = pack_po(f8(SW2 * W2_w))
    in_maps_a = []
    for g in range(N_CORES):
        s, q = divmod(g, 4)
        h0 = HPC * q * DH
        w1qk = np.concatenate([W1_w[:, h0:h0 + 192], W1_w[:, D + h0:D + h0 + 192]], 1)
        b1qk = np.broadcast_to(SQK * np.concatenate(
            [W1_b[h0:h0 + 192], W1_b[D + h0:D + h0 + 192]]), (P, 384))
        bv = SV * W1_b[2 * D + h0: 2 * D + h0 + 192]
        b1v = np.zeros((P, 2), np.float32)
        b1v[:, 0] = bv[:P]
        b1v[:64, 1] = bv[P:]
        in_maps_a.append({
            "xT": xT_pk[s],
            "w1qk": pack_po(f8(SQK * w1qk)),
            "b1qk": f32(b1qk),
            "w1v": pack_po(f8(SV * W1_w[:, 2 * D + h0: 2 * D + h0 + 192])),
            "b1v": b1v,
            "w2": w2_pk,
            "xb": pack_po(xb_pk[s][:, q * ROWS:(q + 1) * ROWS]),
        })
    return in_maps_a


def pack_b_inputs(y1T_list, scale_c, shift_c, sel, fc_w, fc_b, proj_w, proj_b):
    f32 = lambda a: np.ascontiguousarray(a, np.float32)
    fcw_r = {}
    pjw_r = {}
    in_maps_b = []
    for g in range(N_CORES):
        s, q = divmod(g, 4)
        idx, gv = sel[s]
        x1T = (y1T_list[g].astype(np.float64) * scale_c[:, None]
               + shift_c[:, None]).astype(np.float32)
        bcomb = (gv[:, None] * proj_b[np.asarray(idx)].astype(np.float64)).sum(0)
        xbv = (SCALE * (x1T.astype(np.float64) + bcomb[:, None])).astype(np.float32)
        im = {
            "x1f8": pack_po(f8(x1T)),
            "xb": pack_po(f32(xbv)),
        }
        for e in range(2):
            ex = int(idx[e])
            if ex not in fcw_r:
                fr = f8(SCALE * fc_w[ex])
                fcw_r[ex] = np.stack([pack_po(fr[:, c * 512:(c + 1) * 512])
                                      for c in range(6)])
            key = (s, e)
            if key not in pjw_r:
                pw = f8(SCALE * float(gv[e]) * proj_w[ex])
                pjw_r[key] = np.ascontiguousarray(
                    pw.reshape(12, 2, P, D).transpose(0, 2, 1, 3))
            im[f"fcw{e}"] = fcw_r[ex]
            im[f"fcb{e}"] = f32(fc_b[ex].reshape(24, P).T)
            im[f"pjw{e}"] = pjw_r[key]
        in_maps_b.append(im)
    return in_maps_b


def kernel(x, W1_w, W1_b, W2_w, W2_b, r_w, r_b, fc_w, fc_b, proj_w, proj_b,
           ln1_w, ln1_b, ln2_w, ln2_b):
    x = np.asarray(x, np.float32)
    W1_w = np.asarray(W1_w, np.float32)
    W1_b = np.asarray(W1_b, np.float32)
    W2_w = np.asarray(W2_w, np.float32)
    W2_b = np.asarray(W2_b, np.float32)
    r_w = np.asarray(r_w, np.float32)
    r_b = np.asarray(r_b, np.float32)
    fc_w = np.asarray(fc_w, np.float32)
    fc_b = np.asarray(fc_b, np.float32)
    proj_w = np.asarray(proj_w, np.float32)
    proj_b = np.asarray(proj_b, np.float32)
    ln1_w = np.asarray(ln1_w, np.float32)
    ln1_b = np.asarray(ln1_b, np.float32)
    ln2_w = np.asarray(ln2_w, np.float32)
    ln2_b = np.asarray(ln2_b, np.float32)
    in_maps_a = pack_a_inputs(x, W1_w, W1_b, W2_w, W2_b)
    res_a = _run("a", in_maps_a)
    y1T = [res_a.results[g]["y1T"] for g in range(N_CORES)]
    stats = [res_a.results[g]["stats"].astype(np.float64)
             .reshape(P, 6, 2).transpose(1, 0, 2).reshape(D, 2)
             for g in range(N_CORES)]

    # global LN1 stats on y1' = 1024*y1 (scalar mean, unbiased var)
    S = sum(st[:, 0].sum() for st in stats)
    SQ = sum(st[:, 1].sum() for st in stats)
    m1 = S / M_TOT
    v1 = (SQ - S * S / M_TOT) / (M_TOT - 1)
    rstd_true = 1.0 / np.sqrt(v1 / (SCALE_A * SCALE_A) + EPS)
    scale_c = ln1_w.astype(np.float64) * rstd_true / SCALE_A
    shift_c = ln1_b.astype(np.float64) - m1 * scale_c

    # router: gate = softmax(mean_n(x1) @ r_w + r_b); top-2 per sample
    sel = []
    for s in range(B):
        ch_sum = sum(stats[s * 4 + q][:, 0] for q in range(4))
        mean_x1 = (ch_sum / N) * scale_c + shift_c
        logits = mean_x1 @ r_w.astype(np.float64) + r_b.astype(np.float64)
        gate = _softmax_np(logits)
        idx = np.argsort(-gate, kind="stable")[:TOP_K]
        sel.append((idx, gate[idx]))

    in_maps_b = pack_b_inputs(y1T, scale_c, shift_c, sel, fc_w, fc_b,
                              proj_w, proj_b)
    res_b = _run("b", in_maps_b)

    # global LN2 stats from per-channel partials (on y2' = 1024*y2)
    stats2 = [res_b.results[g]["stats"].astype(np.float64)
              .reshape(P, 6, 2).transpose(1, 0, 2).reshape(D, 2)
              for g in range(N_CORES)]
    S2 = sum(st[:, 0].sum() for st in stats2)
    SQ2 = sum(st[:, 1].sum() for st in stats2)
    m2 = S2 / M_TOT
    v2 = (SQ2 - S2 * S2 / M_TOT) / (M_TOT - 1)
    rstd2 = 1.0 / np.sqrt(v2 / (SCALE * SCALE) + EPS)
    sc2 = ln2_w.astype(np.float64) * rstd2 / SCALE
    sh2 = ln2_b.astype(np.float64) - m2 * sc2

    out = np.empty((B, N, D), np.float32)
    for g in range(N_CORES):
        s, q = divmod(g, 4)
        y2T = res_b.results[g]["y2T"].astype(np.float64)
        out[s, q * ROWS:(q + 1) * ROWS, :] = \
            (y2T * sc2[:, None] + sh2[:, None]).T.astype(np.float32)
    return out
